# revision 21
# baseline (speedup 1.0000x reference)
"""Trainium2 Bass kernel for BasicBlock(1w4a): quant-act conv3x3 + BN + ReLU.

Data-parallel over 8 NeuronCores (batch 32 -> 8 x 4). Each core packs 2
samples onto the 128 SBUF partitions (64 channels each) and runs the 3x3
conv as shifted matmuls with block-diagonal weights accumulating in PSUM.

Exactness: activations quantize to integers 0..15, weights binarize to +-1.
Both are exact in fp8e4m3, and PSUM accumulates in fp32 (sums bounded well
below 2^24), so the conv is bit-exact. The DoReFa scale (alpha/15) and
BatchNorm fold into a per-channel affine applied by ScalarE as
relu(scale*psum + bias).

v3 layout: the quant frame grid is PACKED at 112 cols/row (no pad columns),
so every input DMA, quant op, and grid write is fully contiguous per
partition (full-size DMA packets; DVE stays in its fast mode). Without pad
columns a tap read at dw=+-1 wraps into the neighboring row at the image
edge; those wrong contributions are cancelled exactly by 4 tiny extra
matmuls per chunk ("edge fixes") whose weights are the negated edge-tap
columns and whose rhs/psum access patterns stride down the edge column
(integer products in fp32 PSUM cancel bit-exactly).

Other structure vs the 100us baseline:
 - quant chain: ACT relu(15x) in place, DVE min+MAGIC in place, DVE's two
   sub(MAGIC)->fp8 writes into the two grid copies (all contiguous).
 - matmuls run tap-outer over the 4 chunks of a 16-row block into a
   4-bank PSUM tile; a post-tile IR pass drops LDWEIGHTS whose weights AP
   matches the previous load (504 -> 126 loads).
 - PSUM drain is one ACT per 16-row block (4D access pattern).

Matmuls per 4-row output chunk:
  3x DoubleRow pairs {(-1,dw),(+1,dw)}  rhs middle-dim step 224 (2 rows)
  1x DoubleRow pair  {(0,-1),(0,+1)}    via a second grid copy at origin
                                        FC2-2 (pair step FC2-HDR %16==0)
  1x normal          {(0,0)}
  2x DoubleRow + 2x normal edge fixes   free dim 4, strided down col 0/111
"""

import os

import numpy as np
import ml_dtypes

import concourse.bass as bass
import concourse.mybir as mybir
import concourse.tile as tile
from concourse import bacc
from concourse.bass_utils import run_bass_kernel_spmd

# ---- problem constants (hardcoded per harness contract) ----
N_CORES = 8
B_FULL = 32
B_SHARD = B_FULL // N_CORES  # 4
C = 64
H = 112
W = 112
BN_EPS = 1e-5

P = 128           # SBUF partitions
GW = 112          # packed grid row width (== W, no pads)
SLICE = 16        # x-rows per input slice / output rows per mm block
NSL = H // SLICE  # 7 slices (= mm blocks) per sample-pair
NMM = 4 * GW      # matmul free dim per chunk (448)
PSW = 512         # fp32 elems per PSUM bank
HDR = 16          # zero header elems (catches tap reads at flat index -1)
TRL = 32          # zero trailer elems (catches tap reads past the grid)
FROWS = H + 2     # frame grid rows: 112 data + top/bottom zero halo
FGRID = FROWS * GW                 # 12768
FC2 = HDR + FGRID + TRL            # copy2 region start; delta 12800 %16==0
FSTORE = FC2 + FGRID

MAGIC = 12582912.0  # 1.5 * 2^23: x+MAGIC-MAGIC rounds to int, half-to-even

# knobs for A/B experiments
DRAIN = os.environ.get("K_DRAIN", "block")      # block | chunk
RELU = os.environ.get("K_RELU", "act")          # act | dve | alt
DEDUP = os.environ.get("K_DEDUP", "1") == "1"
QROWS = int(os.environ.get("K_QROWS", "16"))    # rows per quant op

_cache = {}


def _dedup_ldweights(nc):
    """Remove InstLdweights whose weights AP repeats the previous PE
    weight load (tap-outer matmul order makes runs of 4 identical loads).
    Waits on a removed load migrate to the following instruction; runs
    after TileContext scheduling, before nc.finalize()."""
    removed = 0
    for func in nc.m.functions:
        for blk in func.blocks:
            il = blk.instructions
            last = None
            idx = 0
            while idx < len(il):
                inst = il[idx]
                tn = type(inst).__name__
                if tn == "InstLdweights":
                    sig = (
                        repr(inst.ins[0]),
                        str(getattr(inst, "perf_mode", None)),
                        str(getattr(inst, "is_transpose", None)),
                        str(getattr(inst, "tile_position", None)),
                    )
                    si = inst.sync_info
                    ups = list(si.on_update) if si else []
                    if sig == last and not ups and idx + 1 < len(il):
                        waits = list(si.on_wait) if si else []
                        if waits:
                            nxt = il[idx + 1]
                            nsi = nxt.sync_info
                            nw = waits + (list(nsi.on_wait) if nsi else [])
                            nu = list(nsi.on_update) if nsi else []
                            nxt.sync_info = mybir.SyncInfo(
                                on_wait=nw, on_update=nu)
                        del il[idx]
                        removed += 1
                        continue
                    last = sig
                elif tn != "InstMatmult":
                    eng = getattr(inst, "engine", None)
                    if eng is not None and "PE" in str(eng):
                        last = None  # unknown PE inst: assume clobbered
                idx += 1
    return removed


def _build_nc(variant):
    assert variant == "fp8dr"
    qdt = mybir.dt.float8e4
    DR = mybir.MatmulPerfMode.DoubleRow

    nc = bacc.Bacc(None, target_bir_lowering=False)
    x = nc.dram_tensor("x", [B_SHARD, C, H, W], mybir.dt.float32,
                       kind="ExternalInput")
    scale_d = nc.dram_tensor("scale", [P, 1], mybir.dt.float32,
                             kind="ExternalInput")
    bias_d = nc.dram_tensor("bias", [P, 1], mybir.dt.float32,
                            kind="ExternalInput")
    # 4 DoubleRow pair sets + 1 single (0,0)
    wdr_d = nc.dram_tensor("wdr", [P, 4 * 2 * P], mybir.dt.float8e4,
                           kind="ExternalInput")
    wsg_d = nc.dram_tensor("wsg", [P, 2 * P], mybir.dt.float8e4,
                           kind="ExternalInput")
    # edge-fix weights: 2 DR pair sets {(-1,dw),(+1,dw)} and 2 singles
    # (0,dw), dw in {-1:set0, +1:set1}, all negated
    wfd_d = nc.dram_tensor("wfd", [P, 2 * 2 * P], mybir.dt.float8e4,
                           kind="ExternalInput")
    wfs_d = nc.dram_tensor("wfs", [P, 2 * P], mybir.dt.float8e4,
                           kind="ExternalInput")
    y = nc.dram_tensor("y", [B_SHARD, C, H, W], mybir.dt.float32,
                       kind="ExternalOutput")

    with tile.TileContext(nc) as tc:
        with (
            tc.tile_pool(name="singles", bufs=1) as singles,
            tc.tile_pool(name="raws", bufs=6) as raw_pool,
            tc.tile_pool(name="qgs", bufs=2) as qg_pool,
            tc.tile_pool(name="outs", bufs=6) as out_pool,
            tc.tile_pool(name="psums", bufs=2, space="PSUM") as psum_pool,
        ):
            # constants ride the GpSimd ring so the x loads own the sync
            # ring from instruction 0
            wdr_t = singles.tile([P, 4 * 2 * P], mybir.dt.float8e4)
            nc.gpsimd.dma_start(out=wdr_t[:, :], in_=wdr_d[:, :])
            wsg_t = singles.tile([P, 2 * P], mybir.dt.float8e4)
            nc.gpsimd.dma_start(out=wsg_t[:, :], in_=wsg_d[:, :])
            wfd_t = singles.tile([P, 2 * 2 * P], mybir.dt.float8e4)
            nc.gpsimd.dma_start(out=wfd_t[:, :], in_=wfd_d[:, :])
            wfs_t = singles.tile([P, 2 * P], mybir.dt.float8e4)
            nc.gpsimd.dma_start(out=wfs_t[:, :], in_=wfs_d[:, :])
            scale_t = singles.tile([P, 1], mybir.dt.float32)
            nc.gpsimd.dma_start(out=scale_t[:, :], in_=scale_d[:, :])
            bias_t = singles.tile([P, 1], mybir.dt.float32)
            nc.gpsimd.dma_start(out=bias_t[:, :], in_=bias_d[:, :])

            def emit_frame(pair):
                # per-pair fp8 quant frame: [hdr][114x112 grid][trl][copy2]
                qg = qg_pool.tile([P, FSTORE], qdt,
                                  name=f"qg{pair}", tag="qg")
                nc.gpsimd.memset(qg[:, 0:HDR], 0.0)
                nc.gpsimd.memset(qg[:, HDR + FGRID:FC2], 0.0)
                # top/bottom zero-halo rows of both grid copies
                nc.gpsimd.memset(qg[:, HDR:HDR + GW], 0.0)
                nc.gpsimd.memset(
                    qg[:, HDR + (FROWS - 1) * GW:HDR + FROWS * GW], 0.0)
                nc.gpsimd.memset(qg[:, FC2 - 2:FC2 - 2 + GW], 0.0)
                nc.gpsimd.memset(
                    qg[:, FC2 - 2 + (FROWS - 1) * GW:FC2 - 2 + FROWS * GW],
                    0.0)
                return qg

            # input DMA batching: early slices alone (short pipeline
            # head), later slices in one transfer per pair of slices
            _plans = {
                "a": {0: 1, 1: 2, 3: 2, 5: 2},
                "b": {0: 1, 1: 1, 2: 2, 4: 2, 6: 1},
                "c": {i: 1 for i in range(7)},
            }
            DMA_PLAN = _plans[os.environ.get("K_DMAPLAN", "a")]
            raw_cur = {}

            def emit_quant(pair, sl, qg):
                # quantize x rows [16*sl, 16*sl+16) into frame grid rows
                # [16*sl+1, 16*sl+17) of both grid copies (all contiguous)
                x2 = x[2 * pair:2 * pair + 2].rearrange(
                    "s c h w -> (s c) h w")      # [128, H, W]
                r0 = SLICE * sl
                if sl in DMA_PLAN:
                    nsl = DMA_PLAN[sl]
                    rt = raw_pool.tile([P, nsl * SLICE, W],
                                       mybir.dt.float32,
                                       name=f"raw{pair}_{sl}", tag="raw")
                    # packed load: contiguous per partition on both sides
                    nc.sync.dma_start(
                        out=rt[:, :, :],
                        in_=x2[:, r0:r0 + nsl * SLICE, :],
                    )
                    raw_cur[pair] = (sl, rt)
                sl0, rt = raw_cur[pair]
                raw = rt[:, (sl - sl0) * SLICE:(sl - sl0 + 1) * SLICE, :]
                par = (pair * NSL + sl) % 2
                for qi, a in enumerate(range(0, SLICE, QROWS)):
                    b = a + QROWS
                    rawh = raw[:, a:b, :].rearrange("p a b -> p (a b)")
                    # t = max(15*x, 0)   (ACT affine is fp32-exact)
                    if RELU == "act" or (RELU == "alt" and (qi + par) % 2 == 0):
                        nc.scalar.activation(
                            out=rawh, in_=rawh,
                            func=mybir.ActivationFunctionType.Relu,
                            scale=15.0,
                        )
                    else:
                        nc.vector.tensor_scalar(
                            out=rawh, in0=rawh,
                            scalar1=15.0, scalar2=0.0,
                            op0=mybir.AluOpType.mult,
                            op1=mybir.AluOpType.max,
                        )
                    # t = min(t,15) + MAGIC (fp32 add rounds, RNE)
                    nc.vector.tensor_scalar(
                        out=rawh, in0=rawh,
                        scalar1=15.0, scalar2=MAGIC,
                        op0=mybir.AluOpType.min, op1=mybir.AluOpType.add,
                    )
                    # q = t - MAGIC -> integers 0..15, exact in fp8;
                    # contiguous writes into both grid copies
                    g0 = HDR + (r0 + 1 + a) * GW
                    n = (b - a) * GW
                    nc.vector.tensor_scalar(
                        out=qg[:, g0:g0 + n], in0=rawh,
                        scalar1=MAGIC, scalar2=None,
                        op0=mybir.AluOpType.subtract,
                    )
                    g2 = g0 + (FC2 - 2 - HDR)
                    nc.vector.tensor_scalar(
                        out=qg[:, g2:g2 + n], in0=rawh,
                        scalar1=MAGIC, scalar2=None,
                        op0=mybir.AluOpType.subtract,
                    )

            def emit_mm(pair, blk, qg, last=False):
                y2 = y[2 * pair:2 * pair + 2].rearrange(
                    "s c h w -> (s c) h w")
                r0 = SLICE * blk
                chs = [r0 // 4 + c for c in range(4)]
                ps = psum_pool.tile([P, 4 * PSW], mybir.dt.float32,
                                    name=f"ps{pair}_{blk}", tag="ps")
                ot = out_pool.tile([P, SLICE, W], mybir.dt.float32,
                                   name=f"ot{pair}_{blk}", tag="ot")

                def dst(c, off=0):
                    d = ps[:, c * PSW + off:c * PSW + off + NMM]
                    return d

                # tap-outer over the block's 4 chunks so consecutive
                # matmuls share weights (dedup pass drops the reloads);
                # group order alternates per block so the boundary load
                # dedups too. start/stop ride the per-chunk end groups.
                def fix_dst(off):
                    d = ps[:, off:off + 3 * PSW + 3 * GW + 1]
                    v = d.ap
                    v[1] = [PSW, 4]
                    v.append([GW, 4])
                    d.ap = v
                    return d

                def g_dr(i):
                    # DoubleRow pair {(-1,dw),(+1,dw)}: row step 2*GW,
                    # or (i==3) pair {(0,-1),(0,+1)} via copy2
                    def f(start, stop):
                        lhsT = wdr_t[:, i * 2 * P:(i + 1) * 2 * P] \
                            .rearrange("p (a b) -> p a b", a=2)
                        for c, ch in enumerate(chs):
                            if i < 3:
                                base = HDR + (4 * ch) * GW + (i - 1)
                                step = 2 * GW
                            else:
                                base = HDR + (4 * ch + 1) * GW - 1
                                step = FC2 - HDR
                            rhs = qg[:, base:base + NMM]
                            v = rhs.ap
                            v[1] = [step, 2]
                            v.append([1, NMM])
                            rhs.ap = v
                            nc.tensor.matmul(
                                dst(c), lhsT=lhsT, rhs=rhs,
                                start=start, stop=stop, perf_mode=DR,
                            )
                    return f

                def g_fd(s):
                    # edge-fix DR, batched over the block's 16 rows:
                    # F- (s=0), out col 0:  -sum_dh w[dh,-1]*G(k+dh, 111)
                    # F+ (s=1), out col 111: -sum_dh w[dh,+1]*G(k+2+dh, 0)
                    def f(start, stop):
                        assert not (start or stop)
                        lhsT = wfd_t[:, s * 2 * P:(s + 1) * 2 * P] \
                            .rearrange("p (a b) -> p a b", a=2)
                        if s == 0:
                            base = HDR + (r0 - 1) * GW + (GW - 1)
                            off = 0
                        else:
                            base = HDR + (r0 + 1) * GW
                            off = GW - 1
                        rhs = qg[:, base:base + NMM]
                        v = rhs.ap
                        v[1] = [2 * GW, 2]
                        v.append([GW, SLICE])
                        rhs.ap = v
                        nc.tensor.matmul(
                            fix_dst(off), lhsT=lhsT, rhs=rhs,
                            start=False, stop=False, perf_mode=DR,
                        )
                    return f

                def g_fs(s):
                    # edge-fix singles (dh=0 term)
                    def f(start, stop):
                        assert not (start or stop)
                        lhsT = wfs_t[:, s * P:(s + 1) * P]
                        if s == 0:
                            base = HDR + r0 * GW + (GW - 1)
                            off = 0
                        else:
                            base = HDR + (r0 + 2) * GW
                            off = GW - 1
                        rhs = qg[:, base:base + NMM]
                        v = rhs.ap
                        v[1] = [GW, SLICE]
                        rhs.ap = v
                        nc.tensor.matmul(
                            fix_dst(off), lhsT=lhsT, rhs=rhs,
                            start=False, stop=False,
                        )
                    return f

                def g_sg(start, stop):
                    # tap (0,0) as a DoubleRow pair with an all-zero
                    # second weight row (0*garbage==0) -> 0.5 cyc/col
                    lhsT = wsg_t.rearrange("p (a b) -> p a b", a=2)
                    for c, ch in enumerate(chs):
                        base = HDR + (4 * ch + 1) * GW
                        rhs = qg[:, base:base + NMM]
                        v = rhs.ap
                        v[1] = [2 * GW, 2]
                        v.append([1, NMM])
                        rhs.ap = v
                        nc.tensor.matmul(
                            dst(c), lhsT=lhsT, rhs=rhs,
                            start=start, stop=stop, perf_mode=DR,
                        )

                groups = [g_dr(0), g_dr(1), g_dr(2), g_dr(3),
                          g_fd(0), g_fd(1), g_fs(0), g_fs(1), g_sg]
                for gi, g in enumerate(groups):
                    g(gi == 0, gi == len(groups) - 1)

                # drain: relu(scale*psum + bias) -> ot -> HBM
                pv = ps.rearrange("p (b x) -> p b x", b=4)[:, :, 0:NMM] \
                    .rearrange("p b (r c) -> p b r c", c=GW)
                if DRAIN == "block" and not last:
                    ot4 = ot.rearrange("p (b r) c -> p b r c", r=4)
                    nc.scalar.activation(
                        out=ot4, in_=pv,
                        func=mybir.ActivationFunctionType.Relu,
                        bias=bias_t[:, 0:1], scale=scale_t[:, 0:1],
                    )
                    nc.gpsimd.dma_start(
                        out=y2[:, r0:r0 + SLICE, :],
                        in_=ot[:, :, :],
                    )
                else:
                    # per-chunk (also used for the last block so the final
                    # transfers overlap the remaining chunks' ACTs)
                    for c in range(4):
                        nc.scalar.activation(
                            out=ot[:, 4 * c:4 * c + 4, :],
                            in_=pv[:, c, :, :],
                            func=mybir.ActivationFunctionType.Relu,
                            bias=bias_t[:, 0:1], scale=scale_t[:, 0:1],
                        )
                        nc.gpsimd.dma_start(
                            out=y2[:, r0 + 4 * c:r0 + 4 * c + 4, :],
                            in_=ot[:, 4 * c:4 * c + 4, :],
                        )

            # Software-pipelined emission. mm block k reads frame grid rows
            # [16k, 16k+17], i.e. slices k-1, k and the first row of slice
            # k+1 -- so mm(k) is emitted after quant(k+1).
            frames = {pr: None for pr in range(B_SHARD // 2)}
            work = [(pr, sl) for pr in range(B_SHARD // 2)
                    for sl in range(NSL)]
            pending = []
            for pr, sl in work:
                if sl == 0:
                    frames[pr] = emit_frame(pr)
                emit_quant(pr, sl, frames[pr])
                if sl >= 1:
                    pending.append((pr, sl - 1, frames[pr]))
                if sl == NSL - 1:
                    pending.append((pr, sl, frames[pr]))
                while len(pending) > 1:
                    emit_mm(*pending.pop(0))
            for pr_, blk_, fr_ in pending:
                emit_mm(pr_, blk_, fr_,
                        last=(pr_ == B_SHARD // 2 - 1 and blk_ == NSL - 1))

    if DEDUP:
        n = _dedup_ldweights(nc)
        print(f"kernel: deduped {n} InstLdweights")
    nc.finalize()
    return nc


def _blockdiag(blk64):
    out = np.zeros((P, P), dtype=np.float32)
    out[0:64, 0:64] = blk64
    out[64:128, 64:128] = blk64
    return out


def _host_prep(w, gamma, beta, bn_mean, bn_var):
    w = np.asarray(w, dtype=np.float32)
    alpha = np.float32(np.mean(np.abs(w)))
    ws = np.sign(w).astype(np.float32)           # [co, ci, 3, 3]
    inv = (np.asarray(gamma, np.float32)
           / np.sqrt(np.asarray(bn_var, np.float32) + np.float32(BN_EPS)))
    scale_c = (inv * (alpha / np.float32(15.0))).astype(np.float32)
    bias_c = (np.asarray(beta, np.float32)
              - np.asarray(bn_mean, np.float32) * inv).astype(np.float32)
    scale128 = np.ascontiguousarray(
        np.concatenate([scale_c, scale_c]).reshape(P, 1))
    bias128 = np.ascontiguousarray(
        np.concatenate([bias_c, bias_c]).reshape(P, 1))

    # pair sets: [(dh=-1,dw),(dh=+1,dw)] for dw in 0..2, then
    # [(0,-1),(0,+1)]; single = (0,0).  w index [co, ci, dh+1, dw+1].
    wdr = np.zeros((P, 4, 2, P), dtype=np.float32)
    for i, dw in enumerate(range(3)):
        wdr[:, i, 0, :] = _blockdiag(ws[:, :, 0, dw].T)  # dh=-1
        wdr[:, i, 1, :] = _blockdiag(ws[:, :, 2, dw].T)  # dh=+1
    wdr[:, 3, 0, :] = _blockdiag(ws[:, :, 1, 0].T)       # (0,-1)
    wdr[:, 3, 1, :] = _blockdiag(ws[:, :, 1, 2].T)       # (0,+1)
    # (0,0) as a DR pair with an all-zero second row
    wsg = np.zeros((P, 2, P), dtype=np.float32)
    wsg[:, 0, :] = _blockdiag(ws[:, :, 1, 1].T)

    # edge-fix weights (negated): set s=0 -> dw=-1 (w col 0),
    # s=1 -> dw=+1 (w col 2); DR pairs {dh=-1,dh=+1}, singles dh=0
    wfd = np.zeros((P, 2, 2, P), dtype=np.float32)
    wfs = np.zeros((P, 2, P), dtype=np.float32)
    for s, wcol in enumerate((0, 2)):
        wfd[:, s, 0, :] = _blockdiag(-ws[:, :, 0, wcol].T)  # dh=-1
        wfd[:, s, 1, :] = _blockdiag(-ws[:, :, 2, wcol].T)  # dh=+1
        wfs[:, s, :] = _blockdiag(-ws[:, :, 1, wcol].T)     # dh=0

    wm = {
        "wdr": np.ascontiguousarray(
            wdr.reshape(P, 8 * P).astype(ml_dtypes.float8_e4m3)),
        "wsg": np.ascontiguousarray(
            wsg.reshape(P, 2 * P).astype(ml_dtypes.float8_e4m3)),
        "wfd": np.ascontiguousarray(
            wfd.reshape(P, 4 * P).astype(ml_dtypes.float8_e4m3)),
        "wfs": np.ascontiguousarray(
            wfs.reshape(P, 2 * P).astype(ml_dtypes.float8_e4m3)),
    }
    return wm, scale128, bias128


_last_results = None  # test harness peeks at this for profile data


def kernel(x, w, gamma, beta, bn_mean, bn_var):
    global _last_results
    variant = "fp8dr"
    if variant not in _cache:
        _cache[variant] = _build_nc(variant)
    nc = _cache[variant]

    wm, scale128, bias128 = _host_prep(w, gamma, beta, bn_mean, bn_var)
    x = np.asarray(x, dtype=np.float32)

    in_maps = []
    for i in range(N_CORES):
        m = {
            "x": np.ascontiguousarray(x[i * B_SHARD:(i + 1) * B_SHARD]),
            "scale": scale128,
            "bias": bias128,
        }
        m.update(wm)
        in_maps.append(m)
    res = run_bass_kernel_spmd(nc, in_maps, core_ids=list(range(N_CORES)))
    _last_results = res
    return np.concatenate([res.results[i]["y"] for i in range(N_CORES)],
                          axis=0)


# revision 22
# speedup vs baseline: 1.0829x; 1.0829x over previous
"""Trainium2 Bass kernel for BasicBlock(1w4a): quant-act conv3x3 + BN + ReLU.

Data-parallel over 8 NeuronCores (batch 32 -> 8 x 4). Each core packs 2
samples onto the 128 SBUF partitions (64 channels each) and runs the 3x3
conv as shifted matmuls with block-diagonal weights accumulating in PSUM.

Exactness: activations quantize to integers 0..15, weights binarize to +-1.
Both are exact in fp8e4m3, and PSUM accumulates in fp32 (sums bounded well
below 2^24), so the conv is bit-exact. The DoReFa scale (alpha/15) and
BatchNorm fold into a per-channel affine applied by ScalarE as
relu(scale*psum + bias).

v3 layout: the quant frame grid is PACKED at 112 cols/row (no pad columns),
so every input DMA, quant op, and grid write is fully contiguous per
partition (full-size DMA packets; DVE stays in its fast mode). Without pad
columns a tap read at dw=+-1 wraps into the neighboring row at the image
edge; those wrong contributions are cancelled exactly by 4 tiny extra
matmuls per chunk ("edge fixes") whose weights are the negated edge-tap
columns and whose rhs/psum access patterns stride down the edge column
(integer products in fp32 PSUM cancel bit-exactly).

Other structure vs the 100us baseline:
 - quant chain: ACT relu(15x) in place, DVE min+MAGIC in place, DVE's two
   sub(MAGIC)->fp8 writes into the two grid copies (all contiguous).
 - matmuls run tap-outer over the 4 chunks of a 16-row block into a
   4-bank PSUM tile; a post-tile IR pass drops LDWEIGHTS whose weights AP
   matches the previous load (504 -> 126 loads).
 - PSUM drain is one ACT per 16-row block (4D access pattern).

Matmuls per 4-row output chunk:
  3x DoubleRow pairs {(-1,dw),(+1,dw)}  rhs middle-dim step 224 (2 rows)
  1x DoubleRow pair  {(0,-1),(0,+1)}    via a second grid copy at origin
                                        FC2-2 (pair step FC2-HDR %16==0)
  1x normal          {(0,0)}
  2x DoubleRow + 2x normal edge fixes   free dim 4, strided down col 0/111
"""

import os

import numpy as np
import ml_dtypes

import concourse.bass as bass
import concourse.mybir as mybir
import concourse.tile as tile
from concourse import bacc
from concourse.bass_utils import run_bass_kernel_spmd

# ---- problem constants (hardcoded per harness contract) ----
N_CORES = 8
B_FULL = 32
B_SHARD = B_FULL // N_CORES  # 4
C = 64
H = 112
W = 112
BN_EPS = 1e-5

P = 128           # SBUF partitions
GW = 112          # packed grid row width (== W, no pads)
SLICE = 16        # x-rows per input slice / output rows per mm block
NSL = H // SLICE  # 7 slices (= mm blocks) per sample-pair
NMM = 4 * GW      # matmul free dim per chunk (448)
PSW = 512         # fp32 elems per PSUM bank
HDR = 16          # zero header elems (catches tap reads at flat index -1)
TRL = 32          # zero trailer elems (catches tap reads past the grid)
FROWS = H + 2     # frame grid rows: 112 data + top/bottom zero halo
FGRID = FROWS * GW                 # 12768
FC2 = HDR + FGRID + TRL            # copy2 region start; delta 12800 %16==0
FSTORE = FC2 + FGRID

MAGIC = 12582912.0  # 1.5 * 2^23: x+MAGIC-MAGIC rounds to int, half-to-even

# knobs for A/B experiments
DRAIN = os.environ.get("K_DRAIN", "block")      # block | chunk
RELU = os.environ.get("K_RELU", "act")          # act | dve | alt
DEDUP = os.environ.get("K_DEDUP", "1") == "1"
QROWS = int(os.environ.get("K_QROWS", "16"))    # rows per quant op

_cache = {}


def _dedup_ldweights(nc):
    """Remove InstLdweights whose weights AP repeats the previous PE
    weight load (tap-outer matmul order makes runs of 4 identical loads).
    Waits on a removed load migrate to the following instruction; runs
    after TileContext scheduling, before nc.finalize()."""
    removed = 0
    for func in nc.m.functions:
        for blk in func.blocks:
            il = blk.instructions
            last = None
            idx = 0
            while idx < len(il):
                inst = il[idx]
                tn = type(inst).__name__
                if tn == "InstLdweights":
                    sig = (
                        repr(inst.ins[0]),
                        str(getattr(inst, "perf_mode", None)),
                        str(getattr(inst, "is_transpose", None)),
                        str(getattr(inst, "tile_position", None)),
                    )
                    si = inst.sync_info
                    ups = list(si.on_update) if si else []
                    if sig == last and not ups and idx + 1 < len(il):
                        waits = list(si.on_wait) if si else []
                        if waits:
                            nxt = il[idx + 1]
                            nsi = nxt.sync_info
                            nw = waits + (list(nsi.on_wait) if nsi else [])
                            nu = list(nsi.on_update) if nsi else []
                            nxt.sync_info = mybir.SyncInfo(
                                on_wait=nw, on_update=nu)
                        del il[idx]
                        removed += 1
                        continue
                    last = sig
                elif tn != "InstMatmult":
                    eng = getattr(inst, "engine", None)
                    if eng is not None and "PE" in str(eng):
                        last = None  # unknown PE inst: assume clobbered
                idx += 1
    return removed


def _build_nc(variant):
    assert variant == "fp8dr"
    qdt = mybir.dt.float8e4
    DR = mybir.MatmulPerfMode.DoubleRow

    nc = bacc.Bacc(None, target_bir_lowering=False)
    x = nc.dram_tensor("x", [B_SHARD, C, H, W], mybir.dt.float32,
                       kind="ExternalInput")
    scale_d = nc.dram_tensor("scale", [P, 1], mybir.dt.float32,
                             kind="ExternalInput")
    bias_d = nc.dram_tensor("bias", [P, 1], mybir.dt.float32,
                            kind="ExternalInput")
    # 4 DoubleRow pair sets + 1 single (0,0)
    wdr_d = nc.dram_tensor("wdr", [P, 4 * 2 * P], mybir.dt.float8e4,
                           kind="ExternalInput")
    wsg_d = nc.dram_tensor("wsg", [P, 2 * P], mybir.dt.float8e4,
                           kind="ExternalInput")
    # edge-fix weights: 2 DR pair sets {(-1,dw),(+1,dw)} and 2 singles
    # (0,dw), dw in {-1:set0, +1:set1}, all negated
    wfd_d = nc.dram_tensor("wfd", [P, 2 * 2 * P], mybir.dt.float8e4,
                           kind="ExternalInput")
    wfs_d = nc.dram_tensor("wfs", [P, 2 * P], mybir.dt.float8e4,
                           kind="ExternalInput")
    y = nc.dram_tensor("y", [B_SHARD, C, H, W], mybir.dt.float32,
                       kind="ExternalOutput")

    with tile.TileContext(nc) as tc:
        with (
            tc.tile_pool(name="singles", bufs=1) as singles,
            tc.tile_pool(name="raws", bufs=6) as raw_pool,
            tc.tile_pool(name="qgs", bufs=2) as qg_pool,
            tc.tile_pool(name="outs", bufs=6) as out_pool,
            tc.tile_pool(name="psums", bufs=2, space="PSUM") as psum_pool,
        ):
            # constants ride the GpSimd ring so the x loads own the sync
            # ring from instruction 0
            wdr_t = singles.tile([P, 4 * 2 * P], mybir.dt.float8e4)
            nc.gpsimd.dma_start(out=wdr_t[:, :], in_=wdr_d[:, :])
            wsg_t = singles.tile([P, 2 * P], mybir.dt.float8e4)
            nc.gpsimd.dma_start(out=wsg_t[:, :], in_=wsg_d[:, :])
            wfd_t = singles.tile([P, 2 * 2 * P], mybir.dt.float8e4)
            nc.gpsimd.dma_start(out=wfd_t[:, :], in_=wfd_d[:, :])
            wfs_t = singles.tile([P, 2 * P], mybir.dt.float8e4)
            nc.gpsimd.dma_start(out=wfs_t[:, :], in_=wfs_d[:, :])
            scale_t = singles.tile([P, 1], mybir.dt.float32)
            nc.gpsimd.dma_start(out=scale_t[:, :], in_=scale_d[:, :])
            bias_t = singles.tile([P, 1], mybir.dt.float32)
            nc.gpsimd.dma_start(out=bias_t[:, :], in_=bias_d[:, :])

            def emit_frame(pair):
                # per-pair fp8 quant frame: [hdr][114x112 grid][trl][copy2]
                qg = qg_pool.tile([P, FSTORE], qdt,
                                  name=f"qg{pair}", tag="qg")
                nc.gpsimd.memset(qg[:, 0:HDR], 0.0)
                nc.gpsimd.memset(qg[:, HDR + FGRID:FC2], 0.0)
                # top/bottom zero-halo rows of both grid copies
                nc.gpsimd.memset(qg[:, HDR:HDR + GW], 0.0)
                nc.gpsimd.memset(
                    qg[:, HDR + (FROWS - 1) * GW:HDR + FROWS * GW], 0.0)
                nc.gpsimd.memset(qg[:, FC2 - 2:FC2 - 2 + GW], 0.0)
                nc.gpsimd.memset(
                    qg[:, FC2 - 2 + (FROWS - 1) * GW:FC2 - 2 + FROWS * GW],
                    0.0)
                return qg

            # input DMA batching: early slices alone (short pipeline
            # head), later slices in one transfer per pair of slices
            _plans = {
                "a": {0: 1, 1: 2, 3: 2, 5: 2},
                "b": {0: 1, 1: 1, 2: 2, 4: 2, 6: 1},
                "c": {i: 1 for i in range(7)},
            }
            DMA_PLAN = _plans[os.environ.get("K_DMAPLAN", "a")]
            raw_cur = {}

            def emit_quant(pair, sl, qg):
                # quantize x rows [16*sl, 16*sl+16) into frame grid rows
                # [16*sl+1, 16*sl+17) of both grid copies (all contiguous)
                x2 = x[2 * pair:2 * pair + 2].rearrange(
                    "s c h w -> (s c) h w")      # [128, H, W]
                r0 = SLICE * sl
                if sl in DMA_PLAN:
                    nsl = DMA_PLAN[sl]
                    rt = raw_pool.tile([P, nsl * SLICE, W],
                                       mybir.dt.float32,
                                       name=f"raw{pair}_{sl}", tag="raw")
                    # packed load: contiguous per partition on both sides
                    nc.sync.dma_start(
                        out=rt[:, :, :],
                        in_=x2[:, r0:r0 + nsl * SLICE, :],
                    )
                    raw_cur[pair] = (sl, rt)
                sl0, rt = raw_cur[pair]
                raw = rt[:, (sl - sl0) * SLICE:(sl - sl0 + 1) * SLICE, :]
                par = (pair * NSL + sl) % 2
                for qi, a in enumerate(range(0, SLICE, QROWS)):
                    b = a + QROWS
                    rawh = raw[:, a:b, :].rearrange("p a b -> p (a b)")
                    # t = max(15*x, 0)   (ACT affine is fp32-exact)
                    if RELU == "act" or (RELU == "alt" and (qi + par) % 2 == 0):
                        nc.scalar.activation(
                            out=rawh, in_=rawh,
                            func=mybir.ActivationFunctionType.Relu,
                            scale=15.0,
                        )
                    else:
                        nc.vector.tensor_scalar(
                            out=rawh, in0=rawh,
                            scalar1=15.0, scalar2=0.0,
                            op0=mybir.AluOpType.mult,
                            op1=mybir.AluOpType.max,
                        )
                    # t = min(t,15) + MAGIC (fp32 add rounds, RNE)
                    nc.vector.tensor_scalar(
                        out=rawh, in0=rawh,
                        scalar1=15.0, scalar2=MAGIC,
                        op0=mybir.AluOpType.min, op1=mybir.AluOpType.add,
                    )
                    # q = t - MAGIC -> integers 0..15, exact in fp8;
                    # contiguous writes into both grid copies
                    g0 = HDR + (r0 + 1 + a) * GW
                    n = (b - a) * GW
                    nc.vector.tensor_scalar(
                        out=qg[:, g0:g0 + n], in0=rawh,
                        scalar1=MAGIC, scalar2=None,
                        op0=mybir.AluOpType.subtract,
                    )
                    g2 = g0 + (FC2 - 2 - HDR)
                    nc.vector.tensor_scalar(
                        out=qg[:, g2:g2 + n], in0=rawh,
                        scalar1=MAGIC, scalar2=None,
                        op0=mybir.AluOpType.subtract,
                    )

            def emit_mm(pair, blk, qg, last=False):
                y2 = y[2 * pair:2 * pair + 2].rearrange(
                    "s c h w -> (s c) h w")
                r0 = SLICE * blk
                chs = [r0 // 4 + c for c in range(4)]
                ps = psum_pool.tile([P, 4 * PSW], mybir.dt.float32,
                                    name=f"ps{pair}_{blk}", tag="ps")
                ot = out_pool.tile([P, SLICE, W], mybir.dt.float32,
                                   name=f"ot{pair}_{blk}", tag="ot")

                def dst(c, off=0):
                    d = ps[:, c * PSW + off:c * PSW + off + NMM]
                    return d

                # tap-outer over the block's 4 chunks so consecutive
                # matmuls share weights (dedup pass drops the reloads);
                # group order alternates per block so the boundary load
                # dedups too. start/stop ride the per-chunk end groups.
                def fix_dst(off):
                    d = ps[:, off:off + 3 * PSW + 3 * GW + 1]
                    v = d.ap
                    v[1] = [PSW, 4]
                    v.append([GW, 4])
                    d.ap = v
                    return d

                def g_dr(i):
                    # DoubleRow pair {(-1,dw),(+1,dw)}: row step 2*GW,
                    # or (i==3) pair {(0,-1),(0,+1)} via copy2
                    def f(start, stop):
                        lhsT = wdr_t[:, i * 2 * P:(i + 1) * 2 * P] \
                            .rearrange("p (a b) -> p a b", a=2)
                        for c, ch in enumerate(chs):
                            if i < 3:
                                base = HDR + (4 * ch) * GW + (i - 1)
                                step = 2 * GW
                            else:
                                base = HDR + (4 * ch + 1) * GW - 1
                                step = FC2 - HDR
                            rhs = qg[:, base:base + NMM]
                            v = rhs.ap
                            v[1] = [step, 2]
                            v.append([1, NMM])
                            rhs.ap = v
                            nc.tensor.matmul(
                                dst(c), lhsT=lhsT, rhs=rhs,
                                start=start, stop=stop, perf_mode=DR,
                            )
                    return f

                def g_fd(s):
                    # edge-fix DR, batched over the block's 16 rows:
                    # F- (s=0), out col 0:  -sum_dh w[dh,-1]*G(k+dh, 111)
                    # F+ (s=1), out col 111: -sum_dh w[dh,+1]*G(k+2+dh, 0)
                    def f(start, stop):
                        assert not (start or stop)
                        lhsT = wfd_t[:, s * 2 * P:(s + 1) * 2 * P] \
                            .rearrange("p (a b) -> p a b", a=2)
                        if s == 0:
                            base = HDR + (r0 - 1) * GW + (GW - 1)
                            off = 0
                        else:
                            base = HDR + (r0 + 1) * GW
                            off = GW - 1
                        rhs = qg[:, base:base + NMM]
                        v = rhs.ap
                        v[1] = [2 * GW, 2]
                        v.append([GW, SLICE])
                        rhs.ap = v
                        nc.tensor.matmul(
                            fix_dst(off), lhsT=lhsT, rhs=rhs,
                            start=False, stop=False, perf_mode=DR,
                        )
                    return f

                def g_fs(s):
                    # edge-fix singles (dh=0 term)
                    def f(start, stop):
                        assert not (start or stop)
                        lhsT = wfs_t[:, s * P:(s + 1) * P]
                        if s == 0:
                            base = HDR + r0 * GW + (GW - 1)
                            off = 0
                        else:
                            base = HDR + (r0 + 2) * GW
                            off = GW - 1
                        rhs = qg[:, base:base + NMM]
                        v = rhs.ap
                        v[1] = [GW, SLICE]
                        rhs.ap = v
                        nc.tensor.matmul(
                            fix_dst(off), lhsT=lhsT, rhs=rhs,
                            start=False, stop=False,
                        )
                    return f

                def g_sg(start, stop):
                    # normal matmul: tap (0,0)
                    for c, ch in enumerate(chs):
                        base = HDR + (4 * ch + 1) * GW
                        nc.tensor.matmul(
                            dst(c), lhsT=wsg_t[:, 0:P],
                            rhs=qg[:, base:base + NMM],
                            start=start, stop=stop,
                        )

                groups = [g_dr(0), g_dr(1), g_dr(2), g_dr(3),
                          g_fd(0), g_fd(1), g_fs(0), g_fs(1), g_sg]
                for gi, g in enumerate(groups):
                    g(gi == 0, gi == len(groups) - 1)

                # drain: relu(scale*psum + bias) -> ot -> HBM
                pv = ps.rearrange("p (b x) -> p b x", b=4)[:, :, 0:NMM] \
                    .rearrange("p b (r c) -> p b r c", c=GW)
                if DRAIN == "block" and not last:
                    ot4 = ot.rearrange("p (b r) c -> p b r c", r=4)
                    nc.scalar.activation(
                        out=ot4, in_=pv,
                        func=mybir.ActivationFunctionType.Relu,
                        bias=bias_t[:, 0:1], scale=scale_t[:, 0:1],
                    )
                    nc.gpsimd.dma_start(
                        out=y2[:, r0:r0 + SLICE, :],
                        in_=ot[:, :, :],
                    )
                else:
                    # per-chunk (also used for the last block so the final
                    # transfers overlap the remaining chunks' ACTs)
                    for c in range(4):
                        nc.scalar.activation(
                            out=ot[:, 4 * c:4 * c + 4, :],
                            in_=pv[:, c, :, :],
                            func=mybir.ActivationFunctionType.Relu,
                            bias=bias_t[:, 0:1], scale=scale_t[:, 0:1],
                        )
                        nc.gpsimd.dma_start(
                            out=y2[:, r0 + 4 * c:r0 + 4 * c + 4, :],
                            in_=ot[:, 4 * c:4 * c + 4, :],
                        )

            # Software-pipelined emission. mm block k reads frame grid rows
            # [16k, 16k+17], i.e. slices k-1, k and the first row of slice
            # k+1 -- so mm(k) is emitted after quant(k+1).
            frames = {pr: None for pr in range(B_SHARD // 2)}
            work = [(pr, sl) for pr in range(B_SHARD // 2)
                    for sl in range(NSL)]
            pending = []
            for pr, sl in work:
                if sl == 0:
                    frames[pr] = emit_frame(pr)
                emit_quant(pr, sl, frames[pr])
                if sl >= 1:
                    pending.append((pr, sl - 1, frames[pr]))
                if sl == NSL - 1:
                    pending.append((pr, sl, frames[pr]))
                while len(pending) > 1:
                    emit_mm(*pending.pop(0))
            for pr_, blk_, fr_ in pending:
                emit_mm(pr_, blk_, fr_,
                        last=(pr_ == B_SHARD // 2 - 1 and blk_ == NSL - 1))

    if DEDUP:
        n = _dedup_ldweights(nc)
        print(f"kernel: deduped {n} InstLdweights")
    nc.finalize()
    return nc


def _blockdiag(blk64):
    out = np.zeros((P, P), dtype=np.float32)
    out[0:64, 0:64] = blk64
    out[64:128, 64:128] = blk64
    return out


def _host_prep(w, gamma, beta, bn_mean, bn_var):
    w = np.asarray(w, dtype=np.float32)
    alpha = np.float32(np.mean(np.abs(w)))
    ws = np.sign(w).astype(np.float32)           # [co, ci, 3, 3]
    inv = (np.asarray(gamma, np.float32)
           / np.sqrt(np.asarray(bn_var, np.float32) + np.float32(BN_EPS)))
    scale_c = (inv * (alpha / np.float32(15.0))).astype(np.float32)
    bias_c = (np.asarray(beta, np.float32)
              - np.asarray(bn_mean, np.float32) * inv).astype(np.float32)
    scale128 = np.ascontiguousarray(
        np.concatenate([scale_c, scale_c]).reshape(P, 1))
    bias128 = np.ascontiguousarray(
        np.concatenate([bias_c, bias_c]).reshape(P, 1))

    # pair sets: [(dh=-1,dw),(dh=+1,dw)] for dw in 0..2, then
    # [(0,-1),(0,+1)]; single = (0,0).  w index [co, ci, dh+1, dw+1].
    wdr = np.zeros((P, 4, 2, P), dtype=np.float32)
    for i, dw in enumerate(range(3)):
        wdr[:, i, 0, :] = _blockdiag(ws[:, :, 0, dw].T)  # dh=-1
        wdr[:, i, 1, :] = _blockdiag(ws[:, :, 2, dw].T)  # dh=+1
    wdr[:, 3, 0, :] = _blockdiag(ws[:, :, 1, 0].T)       # (0,-1)
    wdr[:, 3, 1, :] = _blockdiag(ws[:, :, 1, 2].T)       # (0,+1)
    # (0,0) as a DR pair with an all-zero second row
    wsg = np.zeros((P, 2, P), dtype=np.float32)
    wsg[:, 0, :] = _blockdiag(ws[:, :, 1, 1].T)

    # edge-fix weights (negated): set s=0 -> dw=-1 (w col 0),
    # s=1 -> dw=+1 (w col 2); DR pairs {dh=-1,dh=+1}, singles dh=0
    wfd = np.zeros((P, 2, 2, P), dtype=np.float32)
    wfs = np.zeros((P, 2, P), dtype=np.float32)
    for s, wcol in enumerate((0, 2)):
        wfd[:, s, 0, :] = _blockdiag(-ws[:, :, 0, wcol].T)  # dh=-1
        wfd[:, s, 1, :] = _blockdiag(-ws[:, :, 2, wcol].T)  # dh=+1
        wfs[:, s, :] = _blockdiag(-ws[:, :, 1, wcol].T)     # dh=0

    wm = {
        "wdr": np.ascontiguousarray(
            wdr.reshape(P, 8 * P).astype(ml_dtypes.float8_e4m3)),
        "wsg": np.ascontiguousarray(
            wsg.reshape(P, 2 * P).astype(ml_dtypes.float8_e4m3)),
        "wfd": np.ascontiguousarray(
            wfd.reshape(P, 4 * P).astype(ml_dtypes.float8_e4m3)),
        "wfs": np.ascontiguousarray(
            wfs.reshape(P, 2 * P).astype(ml_dtypes.float8_e4m3)),
    }
    return wm, scale128, bias128


_last_results = None  # test harness peeks at this for profile data


def kernel(x, w, gamma, beta, bn_mean, bn_var):
    global _last_results
    variant = "fp8dr"
    if variant not in _cache:
        _cache[variant] = _build_nc(variant)
    nc = _cache[variant]

    wm, scale128, bias128 = _host_prep(w, gamma, beta, bn_mean, bn_var)
    x = np.asarray(x, dtype=np.float32)

    in_maps = []
    for i in range(N_CORES):
        m = {
            "x": np.ascontiguousarray(x[i * B_SHARD:(i + 1) * B_SHARD]),
            "scale": scale128,
            "bias": bias128,
        }
        m.update(wm)
        in_maps.append(m)
    res = run_bass_kernel_spmd(nc, in_maps, core_ids=list(range(N_CORES)))
    _last_results = res
    return np.concatenate([res.results[i]["y"] for i in range(N_CORES)],
                          axis=0)
